# revision 22
# baseline (speedup 1.0000x reference)
"""Trainium2 Bass kernel for the CAM (channel attention) module.

Computes, per batch element b:
    q = x[b].reshape(C, N)                      # C=512, N=4096
    E = q @ q.T                                 # C x C  (symmetric)
    att = softmax(rowmax(E) - E, axis=-1)       # == softmax(-E) row-wise
    out = gamma * (att @ q) + x[b]

Sharding: data-parallel over batch. 16 batch elements -> 2 per NeuronCore
across 8 cores. gamma replicated. No collectives.

Per-core kernel strategy (per batch element):
  1. DMA q into SBUF in natural layout qnat[c_part, n_free] (fp32, exact bits
     are reused for the +x residual, so this tile is never rounded).
  2. Build qT[n_part, c_free] with 128 PE [128x128] transposes; 4 transposes
     share one PSUM bank so a single [128,512] DVE copy drains them (4x fewer
     DVE ops). qT is stored as float32r: the DVE copy rounds, satisfying the
     fp32r-producer rule, and the energy matmul then runs at full PE rate
     (1 cycle/row) instead of fp32's 1/4 rate.
  3. E tiles [128, 512] accumulate in PSUM via fp32r matmuls.
  4. Column-oriented softmax avoids transposing the attention matrix:
     att_T[d, c] = exp(min_c - E[d, c]) / R_c with R_c = sum_d exp(...).
     E is symmetric so min_c (row mins) equals the column-min vector; the
     stored E tile read with d on partitions is already att_T-oriented.
     exp argument <= 0 always, so no overflow; R is clamped before the
     reciprocal so no NaN is possible.
  5. U = exp(min_c - E) in bf16 is the stationary operand of the value
     matmul against a bf16 copy of q (cast on the idle scalar engine).
     gamma/R_c (per output partition) and the +x residual are fused into one
     DVE scalar_tensor_tensor per output chunk; x enters only here, in exact
     fp32, so for gamma == 0 the kernel output is bit-exact x.
"""

import sys

import numpy as np

_REPO = "/opt/trn_rl_repo"
if _REPO not in sys.path:
    sys.path.insert(0, _REPO)

B_TOTAL, C, H, W = 16, 512, 64, 64
N = H * W          # 4096
NCORES = 8
B = B_TOTAL // NCORES  # batches per core = 2
CT = C // 128      # 4 c-tiles
NT = N // 128      # 32 n-tiles
NCH = N // 512     # 8 output column chunks

_cache = {}


def _build_program():
    import concourse.bass as bass
    import concourse.bacc as bacc
    import concourse.mybir as mybir
    import concourse.tile as tile
    from contextlib import ExitStack

    f32 = mybir.dt.float32
    f32r = mybir.dt.float32r
    bf16 = mybir.dt.bfloat16
    AX = mybir.AxisListType
    OP = mybir.AluOpType
    ACT = mybir.ActivationFunctionType

    nc = bacc.Bacc("TRN2", target_bir_lowering=False, debug=False)

    x = nc.dram_tensor("x", [B, C, N], f32, kind="ExternalInput").ap()
    g128 = nc.dram_tensor("gamma128", [128, 1], f32, kind="ExternalInput").ap()
    ident_d = nc.dram_tensor("ident", [128, 128], f32, kind="ExternalInput").ap()
    y = nc.dram_tensor("y", [B, C, N], f32, kind="ExternalOutput").ap()

    with ExitStack() as ctx:
        tc = ctx.enter_context(tile.TileContext(nc))
        const_p = ctx.enter_context(tc.tile_pool(name="const", bufs=1))
        # qnat (fp32 q) and qT (f32r transposed q) alternate through 2 slots;
        # batch b+1's qnat lands in the slot freed by batch b's qT so its DMA
        # overlaps batch b's value-matmul phase.
        big_p = ctx.enter_context(tc.tile_pool(name="big", bufs=2))
        q_p = ctx.enter_context(tc.tile_pool(name="qq", bufs=1))
        qbf_p = ctx.enter_context(tc.tile_pool(name="qbf", bufs=1))
        tmp_p = ctx.enter_context(tc.tile_pool(name="tmp", bufs=2))
        sm_p = ctx.enter_context(tc.tile_pool(name="sm", bufs=2))
        rep_p = ctx.enter_context(tc.tile_pool(name="rep", bufs=1))
        osb_p = ctx.enter_context(tc.tile_pool(name="osb", bufs=8))
        ps = ctx.enter_context(tc.tile_pool(name="ps", bufs=8, space="PSUM"))

        ident = const_p.tile([128, 128], f32, tag="ident")
        nc.sync.dma_start(ident[:], ident_d)
        gam = const_p.tile([128, 1], f32, tag="gam")
        nc.sync.dma_start(gam[:], g128)
        ones128 = const_p.tile([128, 1], bf16, tag="ones128")
        nc.gpsimd.memset(ones128[:], 1.0)
        ones1 = const_p.tile([1, 128], f32, tag="ones1")
        nc.gpsimd.memset(ones1[:], 1.0)

        # warm the PE clock during the initial DMA wait: dummy transposes of
        # the identity keep the ramp/HAM window busy so the first real
        # transposes run at full clock
        warm = ps.tile([128, 512], f32, tag="ps", name="warm")
        for w in range(8):
            nc.tensor.matmul(
                warm[:, 128 * (w % 4):128 * (w % 4 + 1)],
                ident[:],
                ident[:],
                is_transpose=True,
                skip_group_check=True,
            )

        for b in range(B):
            # ---- load q in natural layout, chunked so transposes can
            #      start as soon as the first columns land
            qnat = big_p.tile([128, CT, N], f32, tag="big")
            for t in range(CT):
                for lo, hi in [(0, 128), (128, 512)]:
                    nc.sync.dma_start(
                        qnat[:, t, lo:hi],
                        x[b, 128 * t:128 * (t + 1), lo:hi],
                    )
                for h in range(1, 8):
                    nc.sync.dma_start(
                        qnat[:, t, 512 * h:512 * (h + 1)],
                        x[b, 128 * t:128 * (t + 1), 512 * h:512 * (h + 1)],
                    )

            # ---- build qT[n_part, c_free]; 4 transposes per PSUM bank, one
            #      [128,512] DVE copy per bank (rounds to f32r)
            qt = big_p.tile([128, NT, C], f32r, tag="big")
            for t in range(CT):
                for jq in range(NT // 4):
                    tp4 = ps.tile([128, 512], f32, tag="ps")
                    for i in range(4):
                        j = 4 * jq + i
                        nc.tensor.matmul(
                            tp4[:, 128 * i:128 * (i + 1)],
                            qnat[:, t, 128 * j:128 * (j + 1)],
                            ident[:],
                            is_transpose=True,
                            skip_group_check=True,
                        )
                    nc.vector.tensor_copy(
                        qt[:, 4 * jq:4 * (jq + 1), 128 * t:128 * (t + 1)],
                        tp4[:].rearrange("p (a c) -> p a c", a=4),
                    )

            # ---- bf16 copy of q for the value matmul, on the idle scalar
            #      engine (ACT)
            qbf = qbf_p.tile([128, CT, N], bf16, tag="qbf")
            for t in range(CT):
                nc.scalar.copy(qbf[:, t, :], qnat[:, t, :])

            # ---- energy: E is symmetric, so compute only columns
            #      [lo_t:512] per row-tile (lo capped at 256: narrower f32r
            #      moving operands drop to 1/4 rate) and mirror the missing
            #      [128,128] blocks by transposing the stored ones.
            elo = [0, 128, 256, 256]
            mirrors = {0: [(0, 1), (0, 2), (0, 3)], 1: [(1, 2), (1, 3)]}
            rmins = sm_p.tile([128, CT], f32, tag="rmins")
            colrep_ps = ps.tile([128, C], f32, tag="ps")
            E = [ps.tile([128, C], f32, tag="ps", name=f"Et{t_}")
                 for t_ in range(CT)]
            for t in range(CT):
                Et = E[t]
                for j in range(NT):
                    nc.tensor.matmul(
                        Et[:, elo[t]:C],
                        qt[:, j, 128 * t:128 * (t + 1)],
                        qt[:, j, elo[t]:C],
                        start=(j == 0),
                        stop=(j == NT - 1),
                    )
                # mirror blocks sourced from tile t into later tiles' banks
                # (target regions are disjoint from their MM-written ranges,
                # so this can precede those tiles' accumulation)
                for s, tt in mirrors.get(t, []):
                    blk = sm_p.tile([128, 128], f32, tag="mirror")
                    nc.vector.tensor_copy(
                        blk[:], E[s][:, 128 * tt:128 * (tt + 1)]
                    )
                    nc.tensor.matmul(
                        E[tt][:, 128 * s:128 * (s + 1)],
                        blk[:],
                        ident[:],
                        is_transpose=True,
                        skip_group_check=True,
                    )
                # tile t of E is now complete (its own MMs + any mirrors
                # emitted in earlier iterations): fold its stats immediately
                # so only tile 3's chain trails the energy phase
                nc.vector.tensor_reduce(
                    rmins[:, t:t + 1], E[t][:], axis=AX.X, op=OP.min
                )
                tpm = ps.tile([1, 128], f32, tag="ps")
                nc.tensor.transpose(tpm[:], rmins[:, t:t + 1], ident[:])
                stT = sm_p.tile([1, 128], f32, tag="stT")
                nc.vector.tensor_copy(stT[:], tpm[:])
                nc.tensor.matmul(
                    colrep_ps[:, 128 * t:128 * (t + 1)],
                    ones1[:],
                    stT[:],
                    start=True,
                    stop=True,
                )
            colrep = rep_p.tile([128, C], f32, tag="colrep")
            nc.vector.tensor_copy(colrep[:], colrep_ps[:])

            # ---- U[d, c] = exp(min_c - E[d, c])  (<= 1, no overflow)
            U = q_p.tile([128, CT, C], bf16, tag="qq")
            for t in range(CT):
                tmp = tmp_p.tile([128, C], f32, tag="tmp")
                nc.vector.tensor_tensor(
                    tmp[:], colrep[:], E[t][:], op=OP.subtract
                )
                nc.scalar.activation(U[:, t, :], tmp[:], ACT.Exp)

            # ---- out[c, n] = scale_c * sum_d U[d, c] q[d, n] + x[c, n]
            #      R_c = sum_d U[d, c] (PE ones-reduction) is interleaved
            #      per m so the first value matmuls start sooner;
            #      scale_m = gamma / max(R, tiny) per output partition
            for m in range(CT):
                Rp = ps.tile([128, 1], f32, tag="ps")
                for k in range(CT):
                    nc.tensor.matmul(
                        Rp[:],
                        U[:, k, 128 * m:128 * (m + 1)],
                        ones128[:],
                        start=(k == 0),
                        stop=(k == CT - 1),
                    )
                Rsb = sm_p.tile([128, 1], f32, tag="rsb")
                nc.vector.tensor_scalar_max(Rsb[:], Rp[:], 1e-38)
                rec = sm_p.tile([128, 1], f32, tag="rec")
                nc.vector.reciprocal(rec[:], Rsb[:])
                sc = sm_p.tile([128, 1], f32, tag=f"scale{m}")
                nc.vector.tensor_scalar_mul(sc[:], rec[:], gam[:, 0:1])
                O = []
                for n in range(NCH):
                    On = ps.tile([128, 512], f32, tag="ps")
                    O.append(On)
                for k in range(CT):
                    for n in range(NCH):
                        nc.tensor.matmul(
                            O[n][:],
                            U[:, k, 128 * m:128 * (m + 1)],
                            qbf[:, k, 512 * n:512 * (n + 1)],
                            start=(k == 0),
                            stop=(k == CT - 1),
                            skip_group_check=True,
                        )
                for n in range(NCH):
                    osb = osb_p.tile([128, 512], f32, tag="osb")
                    nc.vector.scalar_tensor_tensor(
                        osb[:],
                        O[n][:],
                        sc[:],
                        qnat[:, m, 512 * n:512 * (n + 1)],
                        op0=OP.mult,
                        op1=OP.add,
                    )
                    nc.sync.dma_start(
                        y[b, 128 * m:128 * (m + 1), 512 * n:512 * (n + 1)],
                        osb[:],
                    )

    nc.compile()
    return nc


def get_program():
    if "nc" not in _cache:
        _cache["nc"] = _build_program()
    return _cache["nc"]


def kernel(x, gamma):
    from concourse.bass_utils import run_bass_kernel_spmd

    nc = get_program()
    xr = np.ascontiguousarray(
        np.asarray(x, dtype=np.float32).reshape(B_TOTAL, C, N)
    )
    g = np.asarray(gamma, dtype=np.float32).reshape(1)
    g128 = np.ascontiguousarray(
        np.broadcast_to(g.reshape(1, 1), (128, 1))
    ).astype(np.float32)
    ident = np.eye(128, dtype=np.float32)
    in_maps = [
        {
            "x": xr[i * B:(i + 1) * B],
            "gamma128": g128,
            "ident": ident,
        }
        for i in range(NCORES)
    ]
    res = run_bass_kernel_spmd(nc, in_maps, list(range(NCORES))).results
    y = np.concatenate([res[i]["y"] for i in range(NCORES)], axis=0)
    return y.reshape(B_TOTAL, C, H, W).astype(np.float32)
